# revision 21
# baseline (speedup 1.0000x reference)
"""Trainium2 Bass kernel for the AttentionConvBlock problem.

Reference computation (per batch b of 8):
    q = relu(conv3x3(x, Wq) + bq); k = relu(conv3x3(x, Wk) + bk)
    v = conv3x3(x, Wv) + bv
    S = q @ k (per-channel [128,128] spatial matmul)
    P = softmax over flattened 16384 entries per (b, c)
    y = P @ v + x

Sharding: data-parallel over batch, one batch per NeuronCore (8 cores).

Per-core plan (fp8 DoubleRow conv core, tightened edges):
  - Convs as 9-shift DoubleRow fp8 matmuls contracting both 128-channel
    input chunks at once (lhsT [128,2,128], rhs [128,2,L]); position
    chunks are flat 3-row runs of the padded image (390 cols, 1
    col/cycle); chunk pairs share each shift's weight load; merged pair
    evacuation (one [128,6,128] fp8 tile, one DMA) halves store DMAs.
  - q/k/v round-trip HBM in fp8, position-major [j, c, i]; the q conv
    runs on an on-chip-transposed image so its output lands transposed.
  - Attention in 8-channel groups, split front/back: front = 8
    S-matmuls, batched exp, row sums, one bf16 all-ones matmul
    broadcasting the global softmax denominators; back = 8 Y-matmuls,
    (psum_y * 1/Z) + x fused on DVE, one batched store.
  - The x residual is loaded JIT per group as bf16 [h, c, w] slices;
    stores are plain DMAs (no prefill, no software-DGE accumulate).
  - Phase A: block-0 convs (k, v, q) with xa->xt transpose jobs woven
    in. Phase B: block-1 convs in q, k, v order, with block-0's full
    attention spread across all slots and block-1 FRONTS woven into the
    v-conv third (their DVE/Scalar work hides under conv matmuls).
    Phase C: block-1 backs only, loads prefetched 2 groups ahead, PSUM
    pools alternating, scale+residual split DVE/Scalar/GpSimd so no
    single engine paces the tail, final stores split across queues.
  - Startup: w0 + first-row minis + w[1:9] spread across all three DMA
    queues, then 8-row xa slices round-robin; later weight blocks ride
    behind in thirds. The scalar queue carries no DMA after startup
    (a trigger's semaphore wait there would stall evac/exp compute).
"""
import os
import sys

sys.path.insert(0, "/opt/trn_rl_repo")
os.environ.setdefault("MYCRO_LOCAL_CACHE", "1")

import numpy as np
import ml_dtypes

B, C, H, W = 8, 256, 128, 128
HW = H * W
N_CORES = 8
PAD = 130            # padded row/col length
PADSZ = PAD * PAD    # 16900 valid bytes per icc copy
PADSTRIDE = 16912    # icc stride, padded to %16 for the DoubleRow AP rule
WSCALE = 2.0 ** 13

# 43 position chunks per conv block: 42 x 3 rows + 1 x 2 rows. Flat
# 390-col runs keep the PE at 1 col/cycle — a 4-D strided rhs (valid
# cols only) measured ~30 cycles of restart penalty per row and lost
# 66us overall, so the 2-col seam junk stays and is never evacuated.
CHUNKS = [(r0, 3) for r0 in range(0, 126, 3)] + [(126, 2)]
GROUPS = [CHUNKS[i : i + 2] for i in range(0, len(CHUNKS), 2)]  # 21 pairs + single

_PROG = None


def _build_program():
    import concourse.bass as bass
    import concourse.tile as tile
    from concourse import bacc, mybir

    dt = mybir.dt
    AF = mybir.ActivationFunctionType
    ALU = mybir.AluOpType
    DR = mybir.MatmulPerfMode.DoubleRow

    nc = bacc.Bacc("TRN2", target_bir_lowering=False, debug=False)

    # [H, C, W] so per-partition(h) runs are contiguous 128-elem rows
    xbf_d = nc.dram_tensor("xbf", [H, C, W], dt.bfloat16, kind="ExternalInput").ap()
    x8n_d = nc.dram_tensor("x8n", [2, 128, PADSTRIDE], dt.float8e4, kind="ExternalInput").ap()
    w_d = nc.dram_tensor("wpack", [54, 128, 256], dt.float8e4, kind="ExternalInput").ap()
    b_d = nc.dram_tensor("bpack", [128, 6], dt.float32, kind="ExternalInput").ap()
    y_d = nc.dram_tensor("y", [H, C, W], dt.bfloat16, kind="ExternalOutput").ap()

    with tile.TileContext(nc) as tc:
        from contextlib import ExitStack

        with ExitStack() as ctx:
            const = ctx.enter_context(tc.tile_pool(name="const", bufs=1))
            xpad_p = ctx.enter_context(tc.tile_pool(name="xpad", bufs=1))
            evac = ctx.enter_context(tc.tile_pool(name="evac", bufs=12))
            qload = ctx.enter_context(tc.tile_pool(name="qload", bufs=4))
            kload = ctx.enter_context(tc.tile_pool(name="kload", bufs=4))
            vload = ctx.enter_context(tc.tile_pool(name="vload", bufs=4))
            xrload = ctx.enter_context(tc.tile_pool(name="xrload", bufs=4))
            att = ctx.enter_context(tc.tile_pool(name="att", bufs=20))
            stat = ctx.enter_context(tc.tile_pool(name="stat", bufs=20))
            outp = ctx.enter_context(tc.tile_pool(name="outp", bufs=3))
            scl = ctx.enter_context(tc.tile_pool(name="scl", bufs=3))
            psum_c = ctx.enter_context(tc.tile_pool(name="psc", bufs=4, space="PSUM"))
            psum_a = ctx.enter_context(tc.tile_pool(name="psa", bufs=3, space="PSUM"))
            psum_z = ctx.enter_context(tc.tile_pool(name="psz", bufs=1, space="PSUM"))
            dram = ctx.enter_context(tc.tile_pool(name="dram", bufs=1, space="DRAM"))

            # ---- constants ----
            w_sb = const.tile([128, 54, 256], dt.float8e4)
            b_sb = const.tile([128, 6], dt.float32)
            nc.scalar.dma_start(out=b_sb[:], in_=b_d)
            ones_bf = const.tile([128, 128], dt.bfloat16)
            nc.vector.memset(ones_bf[:], 1.0)

            # ---- startup uploads ----
            # Order per queue is issue order. The k conv needs w[0..9] and
            # the first xa rows within ~5us, so: tiny w0 + row minis +
            # w[1:9] split across all three queues, then the bulk xa rows,
            # with the v/q-conv weight blocks spliced in thirds behind the
            # early slices (v needed ~67us in, q ~134us in).
            xa = xpad_p.tile([128, 2, PADSTRIDE], dt.float8e4, tag="xa")
            xt = xpad_p.tile([128, 2, PADSTRIDE], dt.float8e4, tag="xt")
            SY, GP, SC = nc.sync, nc.gpsimd, nc.scalar
            queues = [SY, GP, SC]

            def up_w(q, a, b):
                q.dma_start(out=w_sb[:, a:b, :], in_=w_d[a:b].rearrange("t p f -> p t f"))

            def up_xa(q, icc, r0, r1):
                q.dma_start(
                    out=xa[:, icc, r0 * PAD : r1 * PAD],
                    in_=x8n_d[icc, :, r0 * PAD : r1 * PAD],
                )

            up_w(SC, 0, 1)
            up_xa(SY, 0, 0, 5)
            up_xa(GP, 1, 0, 5)
            up_xa(SC, 0, 5, 10)
            up_xa(GP, 1, 5, 10)
            up_w(SY, 1, 5)
            up_w(GP, 5, 9)
            # bulk rows 10..130 in 8-row slices, round-robin so either icc
            # of a given range lands on a different queue; v-conv weights
            # (9:27) in thirds after the fourth slice, q-conv weights
            # (27:54) after the ninth
            ROWSLC = [(10 + 8 * i, min(18 + 8 * i, PAD)) for i in range(15)]
            for s, (r0, r1) in enumerate(ROWSLC):
                for icc in range(2):
                    up_xa(queues[(2 * s + icc) % 3], icc, r0, r1)
                if s == 4:
                    up_w(SY, 9, 15)
                    up_w(GP, 15, 21)
                    up_w(SC, 21, 27)
                if s == 9:
                    up_w(SY, 27, 36)
                    up_w(GP, 36, 45)
                    up_w(SC, 45, 54)

            # xt borders zeroed once; interior filled by castT jobs
            for icc in range(2):
                vt = xt[:, icc, 0:PADSZ].rearrange("p (r c) -> p r c", c=PAD)
                nc.vector.memset(vt[:, 0, :], 0.0)
                nc.vector.memset(vt[:, PAD - 1, :], 0.0)
                nc.vector.memset(vt[:, :, 0:1], 0.0)
                nc.vector.memset(vt[:, :, PAD - 1 : PAD], 0.0)
            castT_jobs = []
            for s in range(16):
                for icc in range(2):

                    def castT(s=s, icc=icc):
                        va = xa[:, icc, 0:PADSZ].rearrange("p (r c) -> p r c", c=PAD)
                        vt = xt[:, icc, 0:PADSZ].rearrange("p (r c) -> p r c", c=PAD)
                        dst = vt[:, 1 : 1 + W, 1 + s * 8 : 9 + s * 8]
                        srcv = va[:, 1 + s * 8 : 9 + s * 8, 1 : 1 + W].rearrange(
                            "p h w -> p w h"
                        )
                        if (s * 2 + icc) % 2 == 0:
                            nc.scalar.activation(out=dst, in_=srcv, func=AF.Copy)
                        else:
                            nc.vector.tensor_copy(out=dst, in_=srcv)

                    castT_jobs.append(castT)

            # ---- HBM round-trip buffers: position-major [j, c, i] ----
            qt_dram = dram.tile([128, C, 128], dt.float8e4, tag="qt")
            k_dram = dram.tile([128, C, 128], dt.float8e4, tag="kd")
            v_dram = dram.tile([128, C, 128], dt.float8e4, tag="vd")
            cv_dram = [k_dram, v_dram, qt_dram]  # cvslot order: k, v, q

            evq = [SY, GP]

            def conv_group(occ, cvslot, gi, chunks):
                # one PSUM bank per chunk; shifts share each weight load.
                # Flat [128, 2, nr*PAD] rhs runs keep the PE at 1 col/cycle
                # (a 4-D valid-cols-only rhs measured ~30 cycles/row of
                # restart penalty); the seam junk is simply never evacuated.
                src = xt if cvslot == 2 else xa
                ps = [
                    psum_c.tile([128, 3, PAD], dt.float32, tag="psc", name=f"psc{ci}")
                    for ci in range(len(chunks))
                ]
                for kk in range(9):
                    dy, dx = kk // 3, kk % 3
                    w3 = w_sb[:, occ * 27 + cvslot * 9 + kk, :].rearrange(
                        "p (two o) -> p two o", two=2
                    )
                    for ci, (r0, nr) in enumerate(chunks):
                        s0 = (r0 + dy) * PAD + dx
                        nc.tensor.matmul(
                            ps[ci][:, 0:nr, :],
                            lhsT=w3,
                            rhs=src[:, :, s0 : s0 + nr * PAD],
                            start=(kk == 0),
                            stop=(kk == 8),
                            perf_mode=DR,
                        )
                # merged pair evacuation: one fp8 tile, one store DMA
                rows = sum(nr for _, nr in chunks)
                ev = evac.tile([128, 6, 128], dt.float8e4, tag="ev")
                ro = 0
                for ci, (r0, nr) in enumerate(chunks):
                    nc.scalar.activation(
                        out=ev[:, ro : ro + nr, :],
                        in_=ps[ci][:, 0:nr, 0:128],
                        func=AF.Identity if cvslot == 1 else AF.Relu,
                        bias=b_sb[:, occ * 3 + cvslot : occ * 3 + cvslot + 1],
                        scale=1.0 / WSCALE,
                    )
                    ro += nr
                r0a = chunks[0][0]
                evq[gi % 2].dma_start(
                    out=cv_dram[cvslot][
                        r0a : r0a + rows, occ * 128 : (occ + 1) * 128, :
                    ].rearrange("j c i -> c j i"),
                    in_=ev[:, 0:rows, :],
                )

            # ---- attention: 8-channel groups, front/back split ----
            # front = S matmuls + exp + row-sums + ones-matmul Z broadcast
            # (needs q, k); back = Y matmuls + fused (y/Z + x) + store
            # (needs v and the x residual slice). Fronts of block occ run
            # as soon as that block's q and k convs are done; backs only
            # need v, so the phase-C tail is just Y+scale+store.
            def att_load_qk(occ, g0):
                c0 = occ * 128 + g0
                qt8 = qload.tile([128, 8, 128], dt.float8e4, tag="qt8")
                nc.sync.dma_start(out=qt8[:], in_=qt_dram[:, c0 : c0 + 8, :])
                k8 = kload.tile([128, 8, 128], dt.float8e4, tag="k8")
                nc.gpsimd.dma_start(out=k8[:], in_=k_dram[:, c0 : c0 + 8, :])
                return qt8, k8

            def att_load_v(occ, g0):
                c0 = occ * 128 + g0
                v8 = vload.tile([128, 8, 128], dt.float8e4, tag="v8")
                nc.sync.dma_start(out=v8[:], in_=v_dram[:, c0 : c0 + 8, :])
                xr8 = xrload.tile([128, 8, 128], dt.bfloat16, tag="xr8")
                nc.gpsimd.dma_start(out=xr8[:], in_=xbf_d[:, c0 : c0 + 8, :])
                return v8, xr8

            def att_front(occ, g0, qk, pool):
                qt8, k8 = qk
                ps_s = [
                    pool[0].tile([128, 4, 128], dt.float32, tag=pool[1], name=f"ps_s{i}")
                    for i in range(2)
                ]
                for j in range(8):
                    nc.tensor.matmul(
                        ps_s[j // 4][:, j % 4, :],
                        lhsT=k8[:, j, :],
                        rhs=qt8[:, j, :],
                        start=True,
                        stop=True,
                    )
                p8 = att.tile([128, 8, 128], dt.bfloat16, tag="p8")
                nc.scalar.activation(out=p8[:, 0:4, :], in_=ps_s[0][:], func=AF.Exp)
                nc.scalar.activation(out=p8[:, 4:8, :], in_=ps_s[1][:], func=AF.Exp)
                cs8 = stat.tile([128, 8], dt.bfloat16, tag="cs8")
                # bf16 partial sums feed a bf16 ones-matmul (halves its
                # LDWEIGHTS); ~0.4% on Z, well inside the error budget
                with nc.allow_low_precision(reason="bf16 softmax-sum broadcast"):
                    nc.vector.reduce_sum(cs8[:], p8[:], axis=mybir.AxisListType.X)
                ps_z = psum_z.tile([128, 8], dt.float32, tag="psz")
                nc.tensor.matmul(
                    ps_z[:], lhsT=ones_bf[:], rhs=cs8[:], start=True, stop=True
                )
                rec8 = stat.tile([128, 8], dt.float32, tag="rec8")
                nc.vector.reciprocal(rec8[:], ps_z[:])
                return p8, rec8

            def att_back(occ, g0, v, fr, pool, split_store, tri=False):
                v8, xr8 = v
                p8, rec8 = fr
                c0 = occ * 128 + g0
                ps_y = [
                    pool[0].tile([128, 4, 128], dt.float32, tag=pool[1], name=f"ps_y{i}")
                    for i in range(2)
                ]
                for j in range(8):
                    nc.tensor.matmul(
                        ps_y[j // 4][:, j % 4, :],
                        lhsT=p8[:, j, :],
                        rhs=v8[:, j, :],
                        start=True,
                        stop=True,
                    )
                out8 = outp.tile([128, 8, 128], dt.bfloat16, tag="out8")
                # (psum_y * 1/Z) + x residual. GpSimd cannot read PSUM, so
                # the fused stt rides DVE; in the phase-C tail (tri=True),
                # where Scalar/GpSimd are otherwise idle, half the channels
                # go Scalar (PSUM copy, per-partition 1/Z scale) + GpSimd
                # (SBUF residual add) to stop DVE pacing the tail.
                ndve = 4 if tri else 8
                for j in range(ndve):
                    nc.vector.scalar_tensor_tensor(
                        out=out8[:, j, :],
                        in0=ps_y[j // 4][:, j % 4, :],
                        scalar=rec8[:, j : j + 1],
                        in1=xr8[:, j, :],
                        op0=ALU.mult,
                        op1=ALU.add,
                    )
                if tri:
                    t4 = scl.tile([128, 4, 128], dt.bfloat16, tag="t4")
                    for j in range(4, 8):
                        nc.scalar.activation(
                            out=t4[:, j - 4, :],
                            in_=ps_y[1][:, j % 4, :],
                            func=AF.Copy,
                            scale=rec8[:, j : j + 1],
                        )
                        nc.gpsimd.tensor_tensor(
                            out=out8[:, j, :],
                            in0=t4[:, j - 4, :],
                            in1=xr8[:, j, :],
                            op=ALU.add,
                        )
                if split_store:
                    sq = [SY, GP, SC, SY]
                    for h in range(4):
                        sq[h].dma_start(
                            out=y_d[:, c0 + 2 * h : c0 + 2 * h + 2, :],
                            in_=out8[:, 2 * h : 2 * h + 2, :],
                        )
                else:
                    evq[(g0 // 8) % 2].dma_start(
                        out=y_d[:, c0 : c0 + 8, :], in_=out8[:]
                    )

            # ---- per-group step factories ----
            def group_steps(occ, pool):
                """Return [loadqk, loadv, front, back] thunks for each group."""
                out = []
                for g0 in range(0, 128, 8):
                    st = {}

                    def loadqk(g0=g0, st=st):
                        st["qk"] = att_load_qk(occ, g0)

                    def loadv(g0=g0, st=st):
                        st["v"] = att_load_v(occ, g0)

                    def front(g0=g0, st=st, pool=pool):
                        st["fr"] = att_front(occ, g0, st["qk"], pool)

                    def back(pool_=None, split=False, tri=False, g0=g0, st=st, pool=pool):
                        att_back(occ, g0, st["v"], st["fr"], pool_ or pool, split, tri)

                    out.append((loadqk, loadv, front, back))
                return out

            # Phase A: block-0 convs (k, v, q). castT jobs run 2-per-slot
            # in the first 16 slots so xt is complete well before the q conv.
            slot_jobs = iter(
                [[castT_jobs[2 * i], castT_jobs[2 * i + 1]] for i in range(16)]
            )
            gi = 0
            for cvslot in (0, 1, 2):
                for grp in GROUPS:
                    conv_group(0, cvslot, gi, grp)
                    gi += 1
                    for job in next(slot_jobs, []):
                        job()

            # Phase B: block-1 convs in q, k, v order. Block-0's full
            # attention (4 steps/group) spreads over all 66 slots; block-1
            # fronts (loadqk+front) run in the v-conv third once block-1's
            # q and k are done.
            b0 = group_steps(0, (psum_a, "psa"))
            b1 = group_steps(1, (psum_a, "psa"))
            b0_flat = [s for g in b0 for s in (g[0], g[1], g[2], g[3])]
            b1_fronts = [s for g in b1 for s in (g[0], g[2])]
            nslots = 3 * len(GROUPS)
            b0_done = b1_done = 0
            slot = 0
            for cvslot in (2, 0, 1):
                for grp in GROUPS:
                    conv_group(1, cvslot, gi, grp)
                    gi += 1
                    slot += 1
                    while b0_done < min((slot * 64) // nslots, len(b0_flat)):
                        b0_flat[b0_done]()
                        b0_done += 1
                    if slot > 2 * len(GROUPS):
                        vslot = slot - 2 * len(GROUPS)
                        while b1_done < min((vslot * 32) // len(GROUPS), 32):
                            b1_fronts[b1_done]()
                            b1_done += 1
            for s in b0_flat[b0_done:]:
                s()
            for s in b1_fronts[b1_done:]:
                s()

            # Phase C: block-1 backs only (Y matmuls + scale/residual +
            # store), loads prefetched two groups ahead, PSUM pools
            # alternating, final stores split across queues.
            pools = [(psum_a, "psa"), (psum_c, "psc")]
            b1[0][1]()
            b1[1][1]()
            for i in range(16):
                b1[i][3](pool_=pools[i % 2], split=(i >= 14), tri=True)
                if i + 2 < 16:
                    b1[i + 2][1]()

    nc.compile()
    return nc


def _get_program():
    global _PROG
    if _PROG is None:
        _PROG = _build_program()
    return _PROG


def _pack_weights(Wq, Wk, Wv):
    # w_d[t, ic, icc*128 + oc], t = occ*27 + cvslot*9 + kk (cvslot: k,v,q).
    # The q conv runs on the TRANSPOSED image with the same (dy,dx) shift
    # arithmetic, so its taps must be packed transposed.
    out = np.zeros((54, 128, 256), np.float32)
    for cvslot, Wcv in ((0, Wk), (1, Wv), (2, np.asarray(Wq).transpose(0, 1, 3, 2))):
        a = np.asarray(Wcv, np.float32) * WSCALE  # [ocg, icg, dy, dx]
        a = a.transpose(2, 3, 1, 0).reshape(9, 2, 128, 2, 128)  # kk,icc,ic,occ,oc
        a = a.transpose(3, 0, 2, 1, 4).reshape(2, 9, 128, 256)  # occ,kk,ic,(icc oc)
        for occ in range(2):
            base = occ * 27 + cvslot * 9
            out[base : base + 9] = a[occ]
    return np.clip(out, -240, 240).astype(ml_dtypes.float8_e4m3)


def _pack_x8(xb):
    # xb [256, 128, 128] fp32 -> padded fp8 natural layout
    x8 = np.asarray(xb, np.float32).astype(ml_dtypes.float8_e4m3)
    x8 = x8.reshape(2, 128, H, W)
    nat = np.zeros((2, 128, PAD, PAD), ml_dtypes.float8_e4m3)
    nat[:, :, 1 : 1 + H, 1 : 1 + W] = x8
    full_n = np.zeros((2, 128, PADSTRIDE), ml_dtypes.float8_e4m3)
    full_n[:, :, :PADSZ] = nat.reshape(2, 128, PADSZ)
    return np.ascontiguousarray(full_n)


def _run(inputs, trace=False, trace_kwargs=None):
    from concourse.bass_utils import run_bass_kernel_spmd

    nc = _get_program()
    x = np.ascontiguousarray(np.asarray(inputs["x"], np.float32))
    wpack = _pack_weights(inputs["Wq"], inputs["Wk"], inputs["Wv"])
    bq = np.asarray(inputs["bq"], np.float32)
    bk = np.asarray(inputs["bk"], np.float32)
    bv = np.asarray(inputs["bv"], np.float32)
    # col = occ*3 + cvslot, cvslot order (k, v, q)
    bpack = np.stack(
        [bk[:128], bv[:128], bq[:128], bk[128:], bv[128:], bq[128:]], axis=1
    )
    bpack = np.ascontiguousarray(bpack, dtype=np.float32)  # [128, 6]

    in_maps = []
    for b in range(N_CORES):
        x8n = _pack_x8(x[b])
        xbf = np.ascontiguousarray(
            x[b].astype(ml_dtypes.bfloat16).transpose(1, 0, 2)
        )
        in_maps.append(
            {"xbf": xbf, "x8n": x8n, "wpack": wpack, "bpack": bpack}
        )
    last_err = None
    for attempt in range(3):
        try:
            res = run_bass_kernel_spmd(
                nc,
                in_maps,
                core_ids=list(range(N_CORES)),
                trace=trace,
                **(trace_kwargs or {}),
            )
            break
        except Exception as e:  # transient device/runtime hiccups
            last_err = e
            if attempt == 2:
                raise
            import time

            time.sleep(5.0)
    out = np.stack(
        [
            np.asarray(res.results[b]["y"], np.float32).transpose(1, 0, 2)
            for b in range(N_CORES)
        ],
        axis=0,
    )
    return out, res


def kernel(**inputs) -> np.ndarray:
    out, _ = _run(inputs, trace=False)
    return out


def kernel_traced(inputs):
    try:
        import axon_shim

        axon_shim.install()
    except Exception:
        pass
    out, res = _run(inputs, trace=True)
    return out, res


# revision 23
# speedup vs baseline: 1.0178x; 1.0178x over previous
"""Trainium2 Bass kernel for the AttentionConvBlock problem.

Reference computation (per batch b of 8):
    q = relu(conv3x3(x, Wq) + bq); k = relu(conv3x3(x, Wk) + bk)
    v = conv3x3(x, Wv) + bv
    S = q @ k (per-channel [128,128] spatial matmul)
    P = softmax over flattened 16384 entries per (b, c)
    y = P @ v + x

Sharding: data-parallel over batch, one batch per NeuronCore (8 cores).

Per-core plan (fp8 DoubleRow conv core, tightened edges):
  - Convs as 9-shift DoubleRow fp8 matmuls contracting both 128-channel
    input chunks at once (lhsT [128,2,128], rhs [128,2,L]); position
    chunks are flat 3-row runs of the padded image (390 cols, 1
    col/cycle); chunk pairs share each shift's weight load; merged pair
    evacuation (one [128,6,128] fp8 tile, one DMA) halves store DMAs.
  - q/k/v round-trip HBM in fp8, position-major [j, c, i]; the q conv
    runs on an on-chip-transposed image so its output lands transposed.
  - Attention in 8-channel groups, split front/back: front = 8
    S-matmuls, batched exp, row sums, one bf16 all-ones matmul
    broadcasting the global softmax denominators; back = 8 Y-matmuls,
    (psum_y * 1/Z) + x fused on DVE, one batched store.
  - The x residual is loaded JIT per group as bf16 [h, c, w] slices;
    stores are plain DMAs (no prefill, no software-DGE accumulate).
  - Phase A: block-0 convs (k, v, q) with xa->xt transpose jobs woven
    in. Phase B: block-1 convs in q, k, v order, with block-0's full
    attention spread across all slots and block-1 FRONTS woven into the
    v-conv third (their DVE/Scalar work hides under conv matmuls).
    Phase C: block-1 backs only — P is pre-scaled by 1/Z in the front
    (hidden under convs), so each back is 8 Y-matmuls plus one batched
    residual add per PSUM bank; loads prefetched 2 groups ahead, PSUM
    pools alternating, final stores split across queues.
  - Startup: w0 + first-row minis + w[1:9] spread across all three DMA
    queues, then 8-row xa slices round-robin; later weight blocks ride
    behind in thirds. The scalar queue carries no DMA after startup
    (a trigger's semaphore wait there would stall evac/exp compute).
"""
import os
import sys

sys.path.insert(0, "/opt/trn_rl_repo")
os.environ.setdefault("MYCRO_LOCAL_CACHE", "1")

import numpy as np
import ml_dtypes

B, C, H, W = 8, 256, 128, 128
HW = H * W
N_CORES = 8
PAD = 130            # padded row/col length
PADSZ = PAD * PAD    # 16900 valid bytes per icc copy
PADSTRIDE = 16912    # icc stride, padded to %16 for the DoubleRow AP rule
WSCALE = 2.0 ** 13

# 43 position chunks per conv block: 42 x 3 rows + 1 x 2 rows. Flat
# 390-col runs keep the PE at 1 col/cycle — a 4-D strided rhs (valid
# cols only) measured ~30 cycles of restart penalty per row and lost
# 66us overall, so the 2-col seam junk stays and is never evacuated.
CHUNKS = [(r0, 3) for r0 in range(0, 126, 3)] + [(126, 2)]
GROUPS = [CHUNKS[i : i + 2] for i in range(0, len(CHUNKS), 2)]  # 21 pairs + single

_PROG = None


def _build_program():
    import concourse.bass as bass
    import concourse.tile as tile
    from concourse import bacc, mybir

    dt = mybir.dt
    AF = mybir.ActivationFunctionType
    ALU = mybir.AluOpType
    DR = mybir.MatmulPerfMode.DoubleRow

    nc = bacc.Bacc("TRN2", target_bir_lowering=False, debug=False)

    # [H, C, W] so per-partition(h) runs are contiguous 128-elem rows
    xbf_d = nc.dram_tensor("xbf", [H, C, W], dt.bfloat16, kind="ExternalInput").ap()
    x8n_d = nc.dram_tensor("x8n", [2, 128, PADSTRIDE], dt.float8e4, kind="ExternalInput").ap()
    w_d = nc.dram_tensor("wpack", [54, 128, 256], dt.float8e4, kind="ExternalInput").ap()
    b_d = nc.dram_tensor("bpack", [128, 6], dt.float32, kind="ExternalInput").ap()
    y_d = nc.dram_tensor("y", [H, C, W], dt.bfloat16, kind="ExternalOutput").ap()

    with tile.TileContext(nc) as tc:
        from contextlib import ExitStack

        with ExitStack() as ctx:
            const = ctx.enter_context(tc.tile_pool(name="const", bufs=1))
            xpad_p = ctx.enter_context(tc.tile_pool(name="xpad", bufs=1))
            evac = ctx.enter_context(tc.tile_pool(name="evac", bufs=12))
            qload = ctx.enter_context(tc.tile_pool(name="qload", bufs=4))
            kload = ctx.enter_context(tc.tile_pool(name="kload", bufs=4))
            vload = ctx.enter_context(tc.tile_pool(name="vload", bufs=4))
            xrload = ctx.enter_context(tc.tile_pool(name="xrload", bufs=4))
            att = ctx.enter_context(tc.tile_pool(name="att", bufs=4))
            att2 = ctx.enter_context(tc.tile_pool(name="att2", bufs=18))
            stat = ctx.enter_context(tc.tile_pool(name="stat", bufs=8))
            outp = ctx.enter_context(tc.tile_pool(name="outp", bufs=3))
            psum_c = ctx.enter_context(tc.tile_pool(name="psc", bufs=4, space="PSUM"))
            psum_a = ctx.enter_context(tc.tile_pool(name="psa", bufs=3, space="PSUM"))
            psum_z = ctx.enter_context(tc.tile_pool(name="psz", bufs=1, space="PSUM"))
            dram = ctx.enter_context(tc.tile_pool(name="dram", bufs=1, space="DRAM"))

            # ---- constants ----
            w_sb = const.tile([128, 54, 256], dt.float8e4)
            b_sb = const.tile([128, 6], dt.float32)
            nc.scalar.dma_start(out=b_sb[:], in_=b_d)
            ones_bf = const.tile([128, 128], dt.bfloat16)
            nc.vector.memset(ones_bf[:], 1.0)

            # ---- startup uploads ----
            # Order per queue is issue order. The k conv needs w[0..9] and
            # the first xa rows within ~5us, so: tiny w0 + row minis +
            # w[1:9] split across all three queues, then the bulk xa rows,
            # with the v/q-conv weight blocks spliced in thirds behind the
            # early slices (v needed ~67us in, q ~134us in).
            xa = xpad_p.tile([128, 2, PADSTRIDE], dt.float8e4, tag="xa")
            xt = xpad_p.tile([128, 2, PADSTRIDE], dt.float8e4, tag="xt")
            SY, GP, SC = nc.sync, nc.gpsimd, nc.scalar
            queues = [SY, GP, SC]

            def up_w(q, a, b):
                q.dma_start(out=w_sb[:, a:b, :], in_=w_d[a:b].rearrange("t p f -> p t f"))

            def up_xa(q, icc, r0, r1):
                q.dma_start(
                    out=xa[:, icc, r0 * PAD : r1 * PAD],
                    in_=x8n_d[icc, :, r0 * PAD : r1 * PAD],
                )

            # Arrival-matched schedule. Per-queue transfers are serial at
            # ~30 GB/s and each trigger costs ~650ns of sequencer time, so:
            # small first minis for a fast first matmul, then 12-row slices
            # paced just ahead of the k conv's ~2 rows/us consumption.
            # SC carries the weights (w0 tiny, w[1:9] split; the v/q blocks
            # ride later, well before their conv phases) plus two xa
            # slices; SY/GP stream the rest, icc0/icc1 mirrored.
            up_w(SC, 0, 1)
            up_xa(SY, 0, 0, 6)
            up_xa(GP, 1, 0, 6)
            up_w(SC, 1, 5)
            up_xa(SY, 1, 6, 18)
            up_xa(GP, 0, 6, 18)
            up_w(SC, 5, 9)
            up_xa(SY, 0, 18, 30)
            up_xa(GP, 1, 18, 30)
            up_xa(SY, 1, 30, 42)
            up_xa(GP, 0, 30, 42)
            up_xa(SC, 0, 42, 54)
            up_xa(SC, 1, 42, 54)
            for i, r0 in enumerate(range(54, 126, 12)):
                r1 = r0 + 12
                up_xa(SY, i % 2, r0, r1)
                up_xa(GP, 1 - i % 2, r0, r1)
                if i == 1:
                    up_w(SC, 9, 27)
            up_xa(SY, 0, 126, 130)
            up_xa(GP, 1, 126, 130)
            up_w(SC, 27, 41)
            up_w(SC, 41, 54)

            # xt borders zeroed once; interior filled by castT jobs
            for icc in range(2):
                vt = xt[:, icc, 0:PADSZ].rearrange("p (r c) -> p r c", c=PAD)
                nc.vector.memset(vt[:, 0, :], 0.0)
                nc.vector.memset(vt[:, PAD - 1, :], 0.0)
                nc.vector.memset(vt[:, :, 0:1], 0.0)
                nc.vector.memset(vt[:, :, PAD - 1 : PAD], 0.0)
            castT_jobs = []
            for s in range(16):
                for icc in range(2):

                    def castT(s=s, icc=icc):
                        va = xa[:, icc, 0:PADSZ].rearrange("p (r c) -> p r c", c=PAD)
                        vt = xt[:, icc, 0:PADSZ].rearrange("p (r c) -> p r c", c=PAD)
                        dst = vt[:, 1 : 1 + W, 1 + s * 8 : 9 + s * 8]
                        srcv = va[:, 1 + s * 8 : 9 + s * 8, 1 : 1 + W].rearrange(
                            "p h w -> p w h"
                        )
                        if (s * 2 + icc) % 2 == 0:
                            nc.scalar.activation(out=dst, in_=srcv, func=AF.Copy)
                        else:
                            nc.vector.tensor_copy(out=dst, in_=srcv)

                    castT_jobs.append(castT)

            # ---- HBM round-trip buffers: position-major [j, c, i] ----
            qt_dram = dram.tile([128, C, 128], dt.float8e4, tag="qt")
            k_dram = dram.tile([128, C, 128], dt.float8e4, tag="kd")
            v_dram = dram.tile([128, C, 128], dt.float8e4, tag="vd")
            cv_dram = [k_dram, v_dram, qt_dram]  # cvslot order: k, v, q

            evq = [SY, GP]

            def conv_group(occ, cvslot, gi, chunks):
                # one PSUM bank per chunk; shifts share each weight load.
                # Flat [128, 2, nr*PAD] rhs runs keep the PE at 1 col/cycle
                # (a 4-D valid-cols-only rhs measured ~30 cycles/row of
                # restart penalty); the seam junk is simply never evacuated.
                src = xt if cvslot == 2 else xa
                ps = [
                    psum_c.tile([128, 3, PAD], dt.float32, tag="psc", name=f"psc{ci}")
                    for ci in range(len(chunks))
                ]
                for kk in range(9):
                    dy, dx = kk // 3, kk % 3
                    w3 = w_sb[:, occ * 27 + cvslot * 9 + kk, :].rearrange(
                        "p (two o) -> p two o", two=2
                    )
                    for ci, (r0, nr) in enumerate(chunks):
                        s0 = (r0 + dy) * PAD + dx
                        nc.tensor.matmul(
                            ps[ci][:, 0:nr, :],
                            lhsT=w3,
                            rhs=src[:, :, s0 : s0 + nr * PAD],
                            start=(kk == 0),
                            stop=(kk == 8),
                            perf_mode=DR,
                        )
                # merged pair evacuation: one fp8 tile, one store DMA
                rows = sum(nr for _, nr in chunks)
                ev = evac.tile([128, 6, 128], dt.float8e4, tag="ev")
                ro = 0
                for ci, (r0, nr) in enumerate(chunks):
                    nc.scalar.activation(
                        out=ev[:, ro : ro + nr, :],
                        in_=ps[ci][:, 0:nr, 0:128],
                        func=AF.Identity if cvslot == 1 else AF.Relu,
                        bias=b_sb[:, occ * 3 + cvslot : occ * 3 + cvslot + 1],
                        scale=1.0 / WSCALE,
                    )
                    ro += nr
                r0a = chunks[0][0]
                evq[gi % 2].dma_start(
                    out=cv_dram[cvslot][
                        r0a : r0a + rows, occ * 128 : (occ + 1) * 128, :
                    ].rearrange("j c i -> c j i"),
                    in_=ev[:, 0:rows, :],
                )

            # ---- attention: 8-channel groups, front/back split ----
            # front = S matmuls + exp + row-sums + ones-matmul Z broadcast
            # (needs q, k); back = Y matmuls + fused (y/Z + x) + store
            # (needs v and the x residual slice). Fronts of block occ run
            # as soon as that block's q and k convs are done; backs only
            # need v, so the phase-C tail is just Y+scale+store.
            def att_load_qk(occ, g0):
                c0 = occ * 128 + g0
                qt8 = qload.tile([128, 8, 128], dt.float8e4, tag="qt8")
                nc.sync.dma_start(out=qt8[:], in_=qt_dram[:, c0 : c0 + 8, :])
                k8 = kload.tile([128, 8, 128], dt.float8e4, tag="k8")
                nc.gpsimd.dma_start(out=k8[:], in_=k_dram[:, c0 : c0 + 8, :])
                return qt8, k8

            def att_load_v(occ, g0):
                c0 = occ * 128 + g0
                v8 = vload.tile([128, 8, 128], dt.float8e4, tag="v8")
                nc.sync.dma_start(out=v8[:], in_=v_dram[:, c0 : c0 + 8, :])
                xr8 = xrload.tile([128, 8, 128], dt.bfloat16, tag="xr8")
                nc.gpsimd.dma_start(out=xr8[:], in_=xbf_d[:, c0 : c0 + 8, :])
                return v8, xr8

            def att_front(occ, g0, qk, pool):
                qt8, k8 = qk
                ps_s = [
                    pool[0].tile([128, 4, 128], dt.float32, tag=pool[1], name=f"ps_s{i}")
                    for i in range(2)
                ]
                for j in range(8):
                    nc.tensor.matmul(
                        ps_s[j // 4][:, j % 4, :],
                        lhsT=k8[:, j, :],
                        rhs=qt8[:, j, :],
                        start=True,
                        stop=True,
                    )
                p8 = att.tile([128, 8, 128], dt.bfloat16, tag="p8")
                nc.scalar.activation(out=p8[:, 0:4, :], in_=ps_s[0][:], func=AF.Exp)
                nc.scalar.activation(out=p8[:, 4:8, :], in_=ps_s[1][:], func=AF.Exp)
                cs8 = stat.tile([128, 8], dt.bfloat16, tag="cs8")
                # bf16 partial sums feed a bf16 ones-matmul (halves its
                # LDWEIGHTS); ~0.4% on Z, well inside the error budget
                with nc.allow_low_precision(reason="bf16 softmax-sum broadcast"):
                    nc.vector.reduce_sum(cs8[:], p8[:], axis=mybir.AxisListType.X)
                ps_z = psum_z.tile([128, 8], dt.float32, tag="psz")
                nc.tensor.matmul(
                    ps_z[:], lhsT=ones_bf[:], rhs=cs8[:], start=True, stop=True
                )
                rec8 = stat.tile([128, 8], dt.float32, tag="rec8")
                nc.vector.reciprocal(rec8[:], ps_z[:])
                # pre-scale P by 1/Z now (hidden under conv matmuls) so the
                # back needs no per-channel scalar — its residual add can be
                # batched per PSUM bank
                p8s = att2.tile([128, 8, 128], dt.bfloat16, tag="p8s")
                for j in range(8):
                    nc.vector.tensor_scalar_mul(
                        p8s[:, j, :], p8[:, j, :], rec8[:, j : j + 1]
                    )
                return p8s

            def att_back(occ, g0, v, fr, pool, split_store):
                v8, xr8 = v
                p8s = fr
                c0 = occ * 128 + g0
                ps_y = [
                    pool[0].tile([128, 4, 128], dt.float32, tag=pool[1], name=f"ps_y{i}")
                    for i in range(2)
                ]
                for j in range(8):
                    nc.tensor.matmul(
                        ps_y[j // 4][:, j % 4, :],
                        lhsT=p8s[:, j, :],
                        rhs=v8[:, j, :],
                        start=True,
                        stop=True,
                    )
                out8 = outp.tile([128, 8, 128], dt.bfloat16, tag="out8")
                # P was pre-scaled by 1/Z, so just psum_y + x, one batched
                # add per PSUM bank (GpSimd cannot read PSUM -> DVE)
                for b in range(2):
                    nc.vector.tensor_tensor(
                        out=out8[:, 4 * b : 4 * b + 4, :],
                        in0=ps_y[b][:],
                        in1=xr8[:, 4 * b : 4 * b + 4, :],
                        op=ALU.add,
                    )
                if split_store:
                    sq = [SY, GP, SC, SY]
                    for h in range(4):
                        sq[h].dma_start(
                            out=y_d[:, c0 + 2 * h : c0 + 2 * h + 2, :],
                            in_=out8[:, 2 * h : 2 * h + 2, :],
                        )
                else:
                    evq[(g0 // 8) % 2].dma_start(
                        out=y_d[:, c0 : c0 + 8, :], in_=out8[:]
                    )

            # ---- per-group step factories ----
            def group_steps(occ, pool):
                """Return [loadqk, loadv, front, back] thunks for each group."""
                out = []
                for g0 in range(0, 128, 8):
                    st = {}

                    def loadqk(g0=g0, st=st):
                        st["qk"] = att_load_qk(occ, g0)

                    def loadv(g0=g0, st=st):
                        st["v"] = att_load_v(occ, g0)

                    def front(g0=g0, st=st, pool=pool):
                        st["fr"] = att_front(occ, g0, st["qk"], pool)

                    def back(pool_=None, split=False, g0=g0, st=st, pool=pool):
                        att_back(occ, g0, st["v"], st["fr"], pool_ or pool, split)

                    out.append((loadqk, loadv, front, back))
                return out

            # Phase A: block-0 convs (k, v, q). castT jobs run 2-per-slot
            # in the first 16 slots so xt is complete well before the q conv.
            slot_jobs = iter(
                [[castT_jobs[2 * i], castT_jobs[2 * i + 1]] for i in range(16)]
            )
            gi = 0
            for cvslot in (0, 1, 2):
                for grp in GROUPS:
                    conv_group(0, cvslot, gi, grp)
                    gi += 1
                    for job in next(slot_jobs, []):
                        job()

            # Phase B: block-1 convs in q, k, v order. Block-0's full
            # attention (4 steps/group) spreads over all 66 slots; block-1
            # fronts (loadqk+front) run in the v-conv third once block-1's
            # q and k are done.
            b0 = group_steps(0, (psum_a, "psa"))
            b1 = group_steps(1, (psum_a, "psa"))
            b0_flat = [s for g in b0 for s in (g[0], g[1], g[2], g[3])]
            b1_fronts = [s for g in b1 for s in (g[0], g[2])]
            nslots = 3 * len(GROUPS)
            b0_done = b1_done = 0
            slot = 0
            for cvslot in (2, 0, 1):
                for grp in GROUPS:
                    conv_group(1, cvslot, gi, grp)
                    gi += 1
                    slot += 1
                    while b0_done < min((slot * 64) // nslots, len(b0_flat)):
                        b0_flat[b0_done]()
                        b0_done += 1
                    if slot > 2 * len(GROUPS):
                        vslot = slot - 2 * len(GROUPS)
                        while b1_done < min((vslot * 32) // len(GROUPS), 32):
                            b1_fronts[b1_done]()
                            b1_done += 1
            for s in b0_flat[b0_done:]:
                s()
            for s in b1_fronts[b1_done:]:
                s()

            # Phase C: block-1 backs only (Y matmuls + scale/residual +
            # store), loads prefetched two groups ahead, PSUM pools
            # alternating, final stores split across queues.
            pools = [(psum_a, "psa"), (psum_c, "psc")]
            b1[0][1]()
            b1[1][1]()
            for i in range(16):
                b1[i][3](pool_=pools[i % 2], split=(i >= 14))
                if i + 2 < 16:
                    b1[i + 2][1]()

    nc.compile()
    return nc


def _get_program():
    global _PROG
    if _PROG is None:
        _PROG = _build_program()
    return _PROG


def _pack_weights(Wq, Wk, Wv):
    # w_d[t, ic, icc*128 + oc], t = occ*27 + cvslot*9 + kk (cvslot: k,v,q).
    # The q conv runs on the TRANSPOSED image with the same (dy,dx) shift
    # arithmetic, so its taps must be packed transposed.
    out = np.zeros((54, 128, 256), np.float32)
    for cvslot, Wcv in ((0, Wk), (1, Wv), (2, np.asarray(Wq).transpose(0, 1, 3, 2))):
        a = np.asarray(Wcv, np.float32) * WSCALE  # [ocg, icg, dy, dx]
        a = a.transpose(2, 3, 1, 0).reshape(9, 2, 128, 2, 128)  # kk,icc,ic,occ,oc
        a = a.transpose(3, 0, 2, 1, 4).reshape(2, 9, 128, 256)  # occ,kk,ic,(icc oc)
        for occ in range(2):
            base = occ * 27 + cvslot * 9
            out[base : base + 9] = a[occ]
    return np.clip(out, -240, 240).astype(ml_dtypes.float8_e4m3)


def _pack_x8(xb):
    # xb [256, 128, 128] fp32 -> padded fp8 natural layout
    x8 = np.asarray(xb, np.float32).astype(ml_dtypes.float8_e4m3)
    x8 = x8.reshape(2, 128, H, W)
    nat = np.zeros((2, 128, PAD, PAD), ml_dtypes.float8_e4m3)
    nat[:, :, 1 : 1 + H, 1 : 1 + W] = x8
    full_n = np.zeros((2, 128, PADSTRIDE), ml_dtypes.float8_e4m3)
    full_n[:, :, :PADSZ] = nat.reshape(2, 128, PADSZ)
    return np.ascontiguousarray(full_n)


def _run(inputs, trace=False, trace_kwargs=None):
    from concourse.bass_utils import run_bass_kernel_spmd

    nc = _get_program()
    x = np.ascontiguousarray(np.asarray(inputs["x"], np.float32))
    wpack = _pack_weights(inputs["Wq"], inputs["Wk"], inputs["Wv"])
    bq = np.asarray(inputs["bq"], np.float32)
    bk = np.asarray(inputs["bk"], np.float32)
    bv = np.asarray(inputs["bv"], np.float32)
    # col = occ*3 + cvslot, cvslot order (k, v, q)
    bpack = np.stack(
        [bk[:128], bv[:128], bq[:128], bk[128:], bv[128:], bq[128:]], axis=1
    )
    bpack = np.ascontiguousarray(bpack, dtype=np.float32)  # [128, 6]

    in_maps = []
    for b in range(N_CORES):
        x8n = _pack_x8(x[b])
        xbf = np.ascontiguousarray(
            x[b].astype(ml_dtypes.bfloat16).transpose(1, 0, 2)
        )
        in_maps.append(
            {"xbf": xbf, "x8n": x8n, "wpack": wpack, "bpack": bpack}
        )
    last_err = None
    for attempt in range(3):
        try:
            res = run_bass_kernel_spmd(
                nc,
                in_maps,
                core_ids=list(range(N_CORES)),
                trace=trace,
                **(trace_kwargs or {}),
            )
            break
        except Exception as e:  # transient device/runtime hiccups
            last_err = e
            if attempt == 2:
                raise
            import time

            time.sleep(5.0)
    out = np.stack(
        [
            np.asarray(res.results[b]["y"], np.float32).transpose(1, 0, 2)
            for b in range(N_CORES)
        ],
        axis=0,
    )
    return out, res


def kernel(**inputs) -> np.ndarray:
    out, _ = _run(inputs, trace=False)
    return out


def kernel_traced(inputs):
    try:
        import axon_shim

        axon_shim.install()
    except Exception:
        pass
    out, res = _run(inputs, trace=True)
    return out, res


# revision 25
# speedup vs baseline: 1.0258x; 1.0078x over previous
"""Trainium2 Bass kernel for the AttentionConvBlock problem.

Reference computation (per batch b of 8):
    q = relu(conv3x3(x, Wq) + bq); k = relu(conv3x3(x, Wk) + bk)
    v = conv3x3(x, Wv) + bv
    S = q @ k (per-channel [128,128] spatial matmul)
    P = softmax over flattened 16384 entries per (b, c)
    y = P @ v + x

Sharding: data-parallel over batch, one batch per NeuronCore (8 cores).

Per-core plan (fp8 DoubleRow conv core, tightened edges):
  - Convs as 9-shift DoubleRow fp8 matmuls contracting both 128-channel
    input chunks at once (lhsT [128,2,128], rhs [128,2,L]); position
    chunks are flat 3-row runs of the padded image (390 cols, 1
    col/cycle); chunk pairs share each shift's weight load; merged pair
    evacuation (one [128,6,128] fp8 tile, one DMA) halves store DMAs.
  - q/k/v round-trip HBM in fp8, position-major [j, c, i]; the q conv
    runs on an on-chip-transposed image so its output lands transposed.
  - Attention in 8-channel groups, split front/back: front = 8
    S-matmuls, batched exp, row sums, one bf16 all-ones matmul
    broadcasting the global softmax denominators; back = 8 Y-matmuls,
    (psum_y * 1/Z) + x fused on DVE, one batched store.
  - The x residual is loaded JIT per group as bf16 [h, c, w] slices;
    stores are plain DMAs (no prefill, no software-DGE accumulate).
  - Phase A: block-0 convs (k, v, q) with xa->xt transpose jobs woven
    in. Phase B: block-1 convs in q, k, v order, with block-0's full
    attention spread across all slots and block-1 FRONTS woven into the
    v-conv third (their DVE/Scalar work hides under conv matmuls).
    Phase C: block-1 backs only — P is pre-scaled by 1/Z in the front
    (hidden under convs), so each back is 8 Y-matmuls plus one batched
    residual add per PSUM bank; loads prefetched 2 groups ahead, PSUM
    pools alternating, final stores split across queues.
  - Startup: w0 + first-row minis + w[1:9] spread across all three DMA
    queues, then 8-row xa slices round-robin; later weight blocks ride
    behind in thirds. The scalar queue carries no DMA after startup
    (a trigger's semaphore wait there would stall evac/exp compute).
"""
import os
import sys

sys.path.insert(0, "/opt/trn_rl_repo")
os.environ.setdefault("MYCRO_LOCAL_CACHE", "1")

import numpy as np
import ml_dtypes

B, C, H, W = 8, 256, 128, 128
HW = H * W
N_CORES = 8
PAD = 130            # padded row/col length
PADSZ = PAD * PAD    # 16900 valid bytes per icc copy
PADSTRIDE = 16912    # icc stride, padded to %16 for the DoubleRow AP rule
WSCALE = 2.0 ** 13

# 43 position chunks per conv block: 42 x 3 rows + 1 x 2 rows. Flat
# 390-col runs keep the PE at 1 col/cycle — a 4-D strided rhs (valid
# cols only) measured ~30 cycles of restart penalty per row and lost
# 66us overall, so the 2-col seam junk stays and is never evacuated.
CHUNKS = [(r0, 3) for r0 in range(0, 126, 3)] + [(126, 2)]
GROUPS = [CHUNKS[i : i + 2] for i in range(0, len(CHUNKS), 2)]  # 21 pairs + single

_PROG = None


def _build_program():
    import concourse.bass as bass
    import concourse.tile as tile
    from concourse import bacc, mybir

    dt = mybir.dt
    AF = mybir.ActivationFunctionType
    ALU = mybir.AluOpType
    DR = mybir.MatmulPerfMode.DoubleRow

    nc = bacc.Bacc("TRN2", target_bir_lowering=False, debug=False)

    # [H, C, W] so per-partition(h) runs are contiguous 128-elem rows
    xbf_d = nc.dram_tensor("xbf", [H, C, W], dt.bfloat16, kind="ExternalInput").ap()
    x8n_d = nc.dram_tensor("x8n", [2, 128, PADSTRIDE], dt.float8e4, kind="ExternalInput").ap()
    w_d = nc.dram_tensor("wpack", [54, 128, 256], dt.float8e4, kind="ExternalInput").ap()
    b_d = nc.dram_tensor("bpack", [128, 6], dt.float32, kind="ExternalInput").ap()
    y_d = nc.dram_tensor("y", [H, C, W], dt.bfloat16, kind="ExternalOutput").ap()

    with tile.TileContext(nc) as tc:
        from contextlib import ExitStack

        with ExitStack() as ctx:
            const = ctx.enter_context(tc.tile_pool(name="const", bufs=1))
            xpad_p = ctx.enter_context(tc.tile_pool(name="xpad", bufs=1))
            evac = ctx.enter_context(tc.tile_pool(name="evac", bufs=12))
            qload = ctx.enter_context(tc.tile_pool(name="qload", bufs=4))
            kload = ctx.enter_context(tc.tile_pool(name="kload", bufs=4))
            vload = ctx.enter_context(tc.tile_pool(name="vload", bufs=4))
            xrload = ctx.enter_context(tc.tile_pool(name="xrload", bufs=20))
            att = ctx.enter_context(tc.tile_pool(name="att", bufs=4))
            att2 = ctx.enter_context(tc.tile_pool(name="att2", bufs=18))
            stat = ctx.enter_context(tc.tile_pool(name="stat", bufs=8))
            outp = ctx.enter_context(tc.tile_pool(name="outp", bufs=3))
            psum_c = ctx.enter_context(tc.tile_pool(name="psc", bufs=4, space="PSUM"))
            psum_a = ctx.enter_context(tc.tile_pool(name="psa", bufs=3, space="PSUM"))
            psum_z = ctx.enter_context(tc.tile_pool(name="psz", bufs=1, space="PSUM"))
            dram = ctx.enter_context(tc.tile_pool(name="dram", bufs=1, space="DRAM"))

            # ---- constants ----
            w_sb = const.tile([128, 54, 256], dt.float8e4)
            b_sb = const.tile([128, 6], dt.float32)
            nc.scalar.dma_start(out=b_sb[:], in_=b_d)
            ones_bf = const.tile([128, 128], dt.bfloat16)
            nc.vector.memset(ones_bf[:], 1.0)

            # ---- startup uploads ----
            # Order per queue is issue order. The k conv needs w[0..9] and
            # the first xa rows within ~5us, so: tiny w0 + row minis +
            # w[1:9] split across all three queues, then the bulk xa rows,
            # with the v/q-conv weight blocks spliced in thirds behind the
            # early slices (v needed ~67us in, q ~134us in).
            xa = xpad_p.tile([128, 2, PADSTRIDE], dt.float8e4, tag="xa")
            xt = xpad_p.tile([128, 2, PADSTRIDE], dt.float8e4, tag="xt")
            SY, GP, SC = nc.sync, nc.gpsimd, nc.scalar
            queues = [SY, GP, SC]

            def up_w(q, a, b):
                q.dma_start(out=w_sb[:, a:b, :], in_=w_d[a:b].rearrange("t p f -> p t f"))

            def up_xa(q, icc, r0, r1):
                q.dma_start(
                    out=xa[:, icc, r0 * PAD : r1 * PAD],
                    in_=x8n_d[icc, :, r0 * PAD : r1 * PAD],
                )

            # Arrival-matched schedule. Per-queue transfers are serial at
            # ~30 GB/s and each trigger costs ~650ns of sequencer time, so:
            # small first minis for a fast first matmul, then 12-row slices
            # paced just ahead of the k conv's ~2 rows/us consumption.
            # SC carries the weights (w0 tiny, w[1:9] split; the v/q blocks
            # ride later, well before their conv phases) plus two xa
            # slices; SY/GP stream the rest, icc0/icc1 mirrored.
            up_w(SC, 0, 1)
            up_xa(SY, 0, 0, 6)
            up_xa(GP, 1, 0, 6)
            up_w(SC, 1, 5)
            up_xa(SY, 1, 6, 18)
            up_xa(GP, 0, 6, 18)
            up_w(SC, 5, 9)
            up_xa(SY, 0, 18, 30)
            up_xa(GP, 1, 18, 30)
            up_xa(SY, 1, 30, 42)
            up_xa(GP, 0, 30, 42)
            up_xa(SC, 0, 42, 54)
            up_xa(SC, 1, 42, 54)
            for i, r0 in enumerate(range(54, 126, 12)):
                r1 = r0 + 12
                up_xa(SY, i % 2, r0, r1)
                up_xa(GP, 1 - i % 2, r0, r1)
                if i == 1:
                    up_w(SC, 9, 27)
            up_xa(SY, 0, 126, 130)
            up_xa(GP, 1, 126, 130)
            up_w(SC, 27, 41)
            up_w(SC, 41, 54)

            # xt borders zeroed once; interior filled by castT jobs
            for icc in range(2):
                vt = xt[:, icc, 0:PADSZ].rearrange("p (r c) -> p r c", c=PAD)
                nc.vector.memset(vt[:, 0, :], 0.0)
                nc.vector.memset(vt[:, PAD - 1, :], 0.0)
                nc.vector.memset(vt[:, :, 0:1], 0.0)
                nc.vector.memset(vt[:, :, PAD - 1 : PAD], 0.0)
            castT_jobs = []
            for s in range(16):
                for icc in range(2):

                    def castT(s=s, icc=icc):
                        va = xa[:, icc, 0:PADSZ].rearrange("p (r c) -> p r c", c=PAD)
                        vt = xt[:, icc, 0:PADSZ].rearrange("p (r c) -> p r c", c=PAD)
                        dst = vt[:, 1 : 1 + W, 1 + s * 8 : 9 + s * 8]
                        srcv = va[:, 1 + s * 8 : 9 + s * 8, 1 : 1 + W].rearrange(
                            "p h w -> p w h"
                        )
                        if (s * 2 + icc) % 2 == 0:
                            nc.scalar.activation(out=dst, in_=srcv, func=AF.Copy)
                        else:
                            nc.vector.tensor_copy(out=dst, in_=srcv)

                    castT_jobs.append(castT)

            # ---- HBM round-trip buffers: position-major [j, c, i] ----
            qt_dram = dram.tile([128, C, 128], dt.float8e4, tag="qt")
            k_dram = dram.tile([128, C, 128], dt.float8e4, tag="kd")
            v_dram = dram.tile([128, C, 128], dt.float8e4, tag="vd")
            cv_dram = [k_dram, v_dram, qt_dram]  # cvslot order: k, v, q

            evq = [SY, GP]

            def conv_group(occ, cvslot, gi, chunks):
                # one PSUM bank per chunk; shifts share each weight load.
                # Flat [128, 2, nr*PAD] rhs runs keep the PE at 1 col/cycle
                # (a 4-D valid-cols-only rhs measured ~30 cycles/row of
                # restart penalty); the seam junk is simply never evacuated.
                src = xt if cvslot == 2 else xa
                ps = [
                    psum_c.tile([128, 3, PAD], dt.float32, tag="psc", name=f"psc{ci}")
                    for ci in range(len(chunks))
                ]
                for kk in range(9):
                    dy, dx = kk // 3, kk % 3
                    w3 = w_sb[:, occ * 27 + cvslot * 9 + kk, :].rearrange(
                        "p (two o) -> p two o", two=2
                    )
                    for ci, (r0, nr) in enumerate(chunks):
                        s0 = (r0 + dy) * PAD + dx
                        nc.tensor.matmul(
                            ps[ci][:, 0:nr, :],
                            lhsT=w3,
                            rhs=src[:, :, s0 : s0 + nr * PAD],
                            start=(kk == 0),
                            stop=(kk == 8),
                            perf_mode=DR,
                        )
                # merged pair evacuation: one fp8 tile, one store DMA
                rows = sum(nr for _, nr in chunks)
                ev = evac.tile([128, 6, 128], dt.float8e4, tag="ev")
                ro = 0
                for ci, (r0, nr) in enumerate(chunks):
                    nc.scalar.activation(
                        out=ev[:, ro : ro + nr, :],
                        in_=ps[ci][:, 0:nr, 0:128],
                        func=AF.Identity if cvslot == 1 else AF.Relu,
                        bias=b_sb[:, occ * 3 + cvslot : occ * 3 + cvslot + 1],
                        scale=1.0 / WSCALE,
                    )
                    ro += nr
                r0a = chunks[0][0]
                evq[gi % 2].dma_start(
                    out=cv_dram[cvslot][
                        r0a : r0a + rows, occ * 128 : (occ + 1) * 128, :
                    ].rearrange("j c i -> c j i"),
                    in_=ev[:, 0:rows, :],
                )

            # ---- attention: 8-channel groups, front/back split ----
            # front = S matmuls + exp + row-sums + ones-matmul Z broadcast
            # (needs q, k); back = Y matmuls + fused (y/Z + x) + store
            # (needs v and the x residual slice). Fronts of block occ run
            # as soon as that block's q and k convs are done; backs only
            # need v, so the phase-C tail is just Y+scale+store.
            def att_load_qk(occ, g0):
                c0 = occ * 128 + g0
                qt8 = qload.tile([128, 8, 128], dt.float8e4, tag="qt8")
                nc.sync.dma_start(out=qt8[:], in_=qt_dram[:, c0 : c0 + 8, :])
                k8 = kload.tile([128, 8, 128], dt.float8e4, tag="k8")
                nc.gpsimd.dma_start(out=k8[:], in_=k_dram[:, c0 : c0 + 8, :])
                # the x residual slice has no conv dependency: load it here
                # (phase B) so the phase-C tail only moves v8 + y stores
                xr8 = xrload.tile([128, 8, 128], dt.bfloat16, tag="xr8")
                (nc.sync if (g0 // 8) % 2 else nc.gpsimd).dma_start(
                    out=xr8[:], in_=xbf_d[:, c0 : c0 + 8, :]
                )
                return qt8, k8, xr8

            vq = [nc.sync, nc.gpsimd, nc.scalar]

            def att_load_v(occ, g0, phc=False):
                c0 = occ * 128 + g0
                v8 = vload.tile([128, 8, 128], dt.float8e4, tag="v8")
                q = vq[(g0 // 8) % 3] if phc else nc.sync
                q.dma_start(out=v8[:], in_=v_dram[:, c0 : c0 + 8, :])
                return v8

            def att_front(occ, g0, qk, pool):
                qt8, k8 = qk
                ps_s = [
                    pool[0].tile([128, 4, 128], dt.float32, tag=pool[1], name=f"ps_s{i}")
                    for i in range(2)
                ]
                for j in range(8):
                    nc.tensor.matmul(
                        ps_s[j // 4][:, j % 4, :],
                        lhsT=k8[:, j, :],
                        rhs=qt8[:, j, :],
                        start=True,
                        stop=True,
                    )
                p8 = att.tile([128, 8, 128], dt.bfloat16, tag="p8")
                nc.scalar.activation(out=p8[:, 0:4, :], in_=ps_s[0][:], func=AF.Exp)
                nc.scalar.activation(out=p8[:, 4:8, :], in_=ps_s[1][:], func=AF.Exp)
                cs8 = stat.tile([128, 8], dt.bfloat16, tag="cs8")
                # bf16 partial sums feed a bf16 ones-matmul (halves its
                # LDWEIGHTS); ~0.4% on Z, well inside the error budget
                with nc.allow_low_precision(reason="bf16 softmax-sum broadcast"):
                    nc.vector.reduce_sum(cs8[:], p8[:], axis=mybir.AxisListType.X)
                ps_z = psum_z.tile([128, 8], dt.float32, tag="psz")
                nc.tensor.matmul(
                    ps_z[:], lhsT=ones_bf[:], rhs=cs8[:], start=True, stop=True
                )
                rec8 = stat.tile([128, 8], dt.float32, tag="rec8")
                nc.vector.reciprocal(rec8[:], ps_z[:])
                # pre-scale P by 1/Z now (hidden under conv matmuls) so the
                # back needs no per-channel scalar — its residual add can be
                # batched per PSUM bank
                p8s = att2.tile([128, 8, 128], dt.bfloat16, tag="p8s")
                for j in range(8):
                    nc.vector.tensor_scalar_mul(
                        p8s[:, j, :], p8[:, j, :], rec8[:, j : j + 1]
                    )
                return p8s

            def att_back(occ, g0, v8, xr8, fr, pool, split_store, phc=False):
                p8s = fr
                c0 = occ * 128 + g0
                ps_y = [
                    pool[0].tile([128, 4, 128], dt.float32, tag=pool[1], name=f"ps_y{i}")
                    for i in range(2)
                ]
                for j in range(8):
                    nc.tensor.matmul(
                        ps_y[j // 4][:, j % 4, :],
                        lhsT=p8s[:, j, :],
                        rhs=v8[:, j, :],
                        start=True,
                        stop=True,
                    )
                out8 = outp.tile([128, 8, 128], dt.bfloat16, tag="out8")
                # P was pre-scaled by 1/Z, so just psum_y + x, one batched
                # add per PSUM bank (GpSimd cannot read PSUM -> DVE)
                for b in range(2):
                    nc.vector.tensor_tensor(
                        out=out8[:, 4 * b : 4 * b + 4, :],
                        in0=ps_y[b][:],
                        in1=xr8[:, 4 * b : 4 * b + 4, :],
                        op=ALU.add,
                    )
                if split_store:
                    sq = [SY, GP, SC, SY]
                    for h in range(4):
                        sq[h].dma_start(
                            out=y_d[:, c0 + 2 * h : c0 + 2 * h + 2, :],
                            in_=out8[:, 2 * h : 2 * h + 2, :],
                        )
                else:
                    # scalar queue only in phase C (it is compute-idle there)
                    q = vq[(g0 // 8 + 1) % 3] if phc else evq[(g0 // 8) % 2]
                    q.dma_start(out=y_d[:, c0 : c0 + 8, :], in_=out8[:])

            # ---- per-group step factories ----
            def group_steps(occ, pool):
                """Return [loadqk, loadv, front, back] thunks for each group."""
                out = []
                for g0 in range(0, 128, 8):
                    st = {}

                    def loadqk(g0=g0, st=st):
                        st["qk"] = att_load_qk(occ, g0)

                    def loadv(phc=False, g0=g0, st=st):
                        st["v"] = att_load_v(occ, g0, phc)

                    def front(g0=g0, st=st, pool=pool):
                        st["fr"] = att_front(occ, g0, st["qk"][:2], pool)

                    def back(pool_=None, split=False, phc=False, g0=g0, st=st, pool=pool):
                        att_back(
                            occ, g0, st["v"], st["qk"][2], st["fr"],
                            pool_ or pool, split, phc,
                        )

                    out.append((loadqk, loadv, front, back))
                return out

            # Phase A: block-0 convs (k, v, q). castT jobs run 2-per-slot
            # in the first 16 slots so xt is complete well before the q conv.
            slot_jobs = iter(
                [[castT_jobs[2 * i], castT_jobs[2 * i + 1]] for i in range(16)]
            )
            gi = 0
            for cvslot in (0, 1, 2):
                for grp in GROUPS:
                    conv_group(0, cvslot, gi, grp)
                    gi += 1
                    for job in next(slot_jobs, []):
                        job()

            # Phase B: block-1 convs in q, k, v order. Block-0's full
            # attention (4 steps/group) spreads over all 66 slots; block-1
            # fronts (loadqk+front) run in the v-conv third once block-1's
            # q and k are done.
            b0 = group_steps(0, (psum_a, "psa"))
            b1 = group_steps(1, (psum_a, "psa"))
            b0_flat = [s for g in b0 for s in (g[0], g[1], g[2], g[3])]
            b1_fronts = [s for g in b1 for s in (g[0], g[2])]
            nslots = 3 * len(GROUPS)
            b0_done = b1_done = 0
            slot = 0
            for cvslot in (2, 0, 1):
                for grp in GROUPS:
                    conv_group(1, cvslot, gi, grp)
                    gi += 1
                    slot += 1
                    while b0_done < min((slot * 64) // nslots, len(b0_flat)):
                        b0_flat[b0_done]()
                        b0_done += 1
                    if slot > 2 * len(GROUPS):
                        vslot = slot - 2 * len(GROUPS)
                        while b1_done < min((vslot * 32) // len(GROUPS), 32):
                            b1_fronts[b1_done]()
                            b1_done += 1
            for s in b0_flat[b0_done:]:
                s()
            for s in b1_fronts[b1_done:]:
                s()

            # Phase C: block-1 backs only (Y matmuls + scale/residual +
            # store), loads prefetched two groups ahead, PSUM pools
            # alternating, final stores split across queues.
            pools = [(psum_a, "psa"), (psum_c, "psc")]
            b1[0][1](phc=True)
            b1[1][1](phc=True)
            for i in range(16):
                b1[i][3](pool_=pools[i % 2], split=(i >= 14), phc=True)
                if i + 2 < 16:
                    b1[i + 2][1](phc=True)

    nc.compile()
    return nc


def _get_program():
    global _PROG
    if _PROG is None:
        _PROG = _build_program()
    return _PROG


def _pack_weights(Wq, Wk, Wv):
    # w_d[t, ic, icc*128 + oc], t = occ*27 + cvslot*9 + kk (cvslot: k,v,q).
    # The q conv runs on the TRANSPOSED image with the same (dy,dx) shift
    # arithmetic, so its taps must be packed transposed.
    out = np.zeros((54, 128, 256), np.float32)
    for cvslot, Wcv in ((0, Wk), (1, Wv), (2, np.asarray(Wq).transpose(0, 1, 3, 2))):
        a = np.asarray(Wcv, np.float32) * WSCALE  # [ocg, icg, dy, dx]
        a = a.transpose(2, 3, 1, 0).reshape(9, 2, 128, 2, 128)  # kk,icc,ic,occ,oc
        a = a.transpose(3, 0, 2, 1, 4).reshape(2, 9, 128, 256)  # occ,kk,ic,(icc oc)
        for occ in range(2):
            base = occ * 27 + cvslot * 9
            out[base : base + 9] = a[occ]
    return np.clip(out, -240, 240).astype(ml_dtypes.float8_e4m3)


def _pack_x8(xb):
    # xb [256, 128, 128] fp32 -> padded fp8 natural layout
    x8 = np.asarray(xb, np.float32).astype(ml_dtypes.float8_e4m3)
    x8 = x8.reshape(2, 128, H, W)
    nat = np.zeros((2, 128, PAD, PAD), ml_dtypes.float8_e4m3)
    nat[:, :, 1 : 1 + H, 1 : 1 + W] = x8
    full_n = np.zeros((2, 128, PADSTRIDE), ml_dtypes.float8_e4m3)
    full_n[:, :, :PADSZ] = nat.reshape(2, 128, PADSZ)
    return np.ascontiguousarray(full_n)


def _run(inputs, trace=False, trace_kwargs=None):
    from concourse.bass_utils import run_bass_kernel_spmd

    nc = _get_program()
    x = np.ascontiguousarray(np.asarray(inputs["x"], np.float32))
    wpack = _pack_weights(inputs["Wq"], inputs["Wk"], inputs["Wv"])
    bq = np.asarray(inputs["bq"], np.float32)
    bk = np.asarray(inputs["bk"], np.float32)
    bv = np.asarray(inputs["bv"], np.float32)
    # col = occ*3 + cvslot, cvslot order (k, v, q)
    bpack = np.stack(
        [bk[:128], bv[:128], bq[:128], bk[128:], bv[128:], bq[128:]], axis=1
    )
    bpack = np.ascontiguousarray(bpack, dtype=np.float32)  # [128, 6]

    in_maps = []
    for b in range(N_CORES):
        x8n = _pack_x8(x[b])
        xbf = np.ascontiguousarray(
            x[b].astype(ml_dtypes.bfloat16).transpose(1, 0, 2)
        )
        in_maps.append(
            {"xbf": xbf, "x8n": x8n, "wpack": wpack, "bpack": bpack}
        )
    last_err = None
    for attempt in range(3):
        try:
            res = run_bass_kernel_spmd(
                nc,
                in_maps,
                core_ids=list(range(N_CORES)),
                trace=trace,
                **(trace_kwargs or {}),
            )
            break
        except Exception as e:  # transient device/runtime hiccups
            last_err = e
            if attempt == 2:
                raise
            import time

            time.sleep(5.0)
    out = np.stack(
        [
            np.asarray(res.results[b]["y"], np.float32).transpose(1, 0, 2)
            for b in range(N_CORES)
        ],
        axis=0,
    )
    return out, res


def kernel(**inputs) -> np.ndarray:
    out, _ = _run(inputs, trace=False)
    return out


def kernel_traced(inputs):
    try:
        import axon_shim

        axon_shim.install()
    except Exception:
        pass
    out, res = _run(inputs, trace=True)
    return out, res


# revision 27
# speedup vs baseline: 1.0282x; 1.0024x over previous
"""Trainium2 Bass kernel for the AttentionConvBlock problem.

Reference computation (per batch b of 8):
    q = relu(conv3x3(x, Wq) + bq); k = relu(conv3x3(x, Wk) + bk)
    v = conv3x3(x, Wv) + bv
    S = q @ k (per-channel [128,128] spatial matmul)
    P = softmax over flattened 16384 entries per (b, c)
    y = P @ v + x

Sharding: data-parallel over batch, one batch per NeuronCore (8 cores).

Per-core plan (fp8 DoubleRow conv core, tightened edges):
  - Convs as 9-shift DoubleRow fp8 matmuls contracting both 128-channel
    input chunks at once (lhsT [128,2,128], rhs [128,2,L]); position
    chunks are flat 3-row runs of the padded image (390 cols, 1
    col/cycle); chunk pairs share each shift's weight load; merged pair
    evacuation (one [128,6,128] fp8 tile, one DMA) halves store DMAs.
  - q/k/v round-trip HBM in fp8, position-major [j, c, i]; the q conv
    runs on an on-chip-transposed image so its output lands transposed.
  - Attention in 8-channel groups, split front/back: front = 8
    S-matmuls, batched exp, row sums, one bf16 all-ones matmul
    broadcasting the global softmax denominators; back = 8 Y-matmuls,
    (psum_y * 1/Z) + x fused on DVE, one batched store.
  - The x residual is loaded JIT per group as bf16 [h, c, w] slices;
    stores are plain DMAs (no prefill, no software-DGE accumulate).
  - Phase A: block-0 convs (k, v, q) with xa->xt transpose jobs woven
    in. Phase B: block-1 convs in q, k, v order, with block-0's full
    attention spread across all slots and block-1 FRONTS woven into the
    v-conv third (their DVE/Scalar work hides under conv matmuls).
    Phase C: block-1 backs only — P is pre-scaled by 1/Z in the front
    (hidden under convs), so each back is 8 Y-matmuls plus one batched
    residual add per PSUM bank; loads prefetched 2 groups ahead, PSUM
    pools alternating, final stores split across queues.
  - Startup: w0 + first-row minis + w[1:9] spread across all three DMA
    queues, then 8-row xa slices round-robin; later weight blocks ride
    behind in thirds. The scalar queue carries no DMA after startup
    (a trigger's semaphore wait there would stall evac/exp compute).
"""
import os
import sys

sys.path.insert(0, "/opt/trn_rl_repo")
os.environ.setdefault("MYCRO_LOCAL_CACHE", "1")

import numpy as np
import ml_dtypes

B, C, H, W = 8, 256, 128, 128
HW = H * W
N_CORES = 8
PAD = 130            # padded row/col length
PADSZ = PAD * PAD    # 16900 valid bytes per icc copy
PADSTRIDE = 16912    # icc stride, padded to %16 for the DoubleRow AP rule
WSCALE = 2.0 ** 13

# 43 position chunks per conv block: 42 x 3 rows + 1 x 2 rows. Flat
# 390-col runs keep the PE at 1 col/cycle — a 4-D strided rhs (valid
# cols only) measured ~30 cycles of restart penalty per row and lost
# 66us overall, so the 2-col seam junk stays and is never evacuated.
CHUNKS = [(r0, 3) for r0 in range(0, 126, 3)] + [(126, 2)]
GROUPS = [CHUNKS[i : i + 2] for i in range(0, len(CHUNKS), 2)]  # 21 pairs + single

_PROG = None


def _build_program():
    import concourse.bass as bass
    import concourse.tile as tile
    from concourse import bacc, mybir

    dt = mybir.dt
    AF = mybir.ActivationFunctionType
    ALU = mybir.AluOpType
    DR = mybir.MatmulPerfMode.DoubleRow

    nc = bacc.Bacc("TRN2", target_bir_lowering=False, debug=False)

    # [H, C, W] so per-partition(h) runs are contiguous 128-elem rows
    xbf_d = nc.dram_tensor("xbf", [H, C, W], dt.bfloat16, kind="ExternalInput").ap()
    x8n_d = nc.dram_tensor("x8n", [2, 128, PADSTRIDE], dt.float8e4, kind="ExternalInput").ap()
    w_d = nc.dram_tensor("wpack", [54, 128, 256], dt.float8e4, kind="ExternalInput").ap()
    b_d = nc.dram_tensor("bpack", [128, 6], dt.float32, kind="ExternalInput").ap()
    y_d = nc.dram_tensor("y", [H, C, W], dt.bfloat16, kind="ExternalOutput").ap()

    with tile.TileContext(nc) as tc:
        from contextlib import ExitStack

        with ExitStack() as ctx:
            const = ctx.enter_context(tc.tile_pool(name="const", bufs=1))
            xpad_p = ctx.enter_context(tc.tile_pool(name="xpad", bufs=1))
            evac = ctx.enter_context(tc.tile_pool(name="evac", bufs=12))
            qload = ctx.enter_context(tc.tile_pool(name="qload", bufs=4))
            kload = ctx.enter_context(tc.tile_pool(name="kload", bufs=4))
            vload = ctx.enter_context(tc.tile_pool(name="vload", bufs=4))
            xrload = ctx.enter_context(tc.tile_pool(name="xrload", bufs=20))
            att = ctx.enter_context(tc.tile_pool(name="att", bufs=4))
            att2 = ctx.enter_context(tc.tile_pool(name="att2", bufs=18))
            stat = ctx.enter_context(tc.tile_pool(name="stat", bufs=8))
            outp = ctx.enter_context(tc.tile_pool(name="outp", bufs=3))
            psum_c = ctx.enter_context(tc.tile_pool(name="psc", bufs=4, space="PSUM"))
            psum_a = ctx.enter_context(tc.tile_pool(name="psa", bufs=3, space="PSUM"))
            psum_z = ctx.enter_context(tc.tile_pool(name="psz", bufs=1, space="PSUM"))
            dram = ctx.enter_context(tc.tile_pool(name="dram", bufs=1, space="DRAM"))

            # ---- constants ----
            w_sb = const.tile([128, 54, 256], dt.float8e4)
            b_sb = const.tile([128, 6], dt.float32)
            ones_bf = const.tile([128, 128], dt.bfloat16)
            nc.vector.memset(ones_bf[:], 1.0)

            # ---- startup uploads ----
            # Order per queue is issue order. The k conv needs w[0..9] and
            # the first xa rows within ~5us, so: tiny w0 + row minis +
            # w[1:9] split across all three queues, then the bulk xa rows,
            # with the v/q-conv weight blocks spliced in thirds behind the
            # early slices (v needed ~67us in, q ~134us in).
            xa = xpad_p.tile([128, 2, PADSTRIDE], dt.float8e4, tag="xa")
            xt = xpad_p.tile([128, 2, PADSTRIDE], dt.float8e4, tag="xt")
            SY, GP, SC = nc.sync, nc.gpsimd, nc.scalar
            queues = [SY, GP, SC]

            def up_w(q, a, b):
                q.dma_start(out=w_sb[:, a:b, :], in_=w_d[a:b].rearrange("t p f -> p t f"))

            def up_xa(q, icc, r0, r1):
                q.dma_start(
                    out=xa[:, icc, r0 * PAD : r1 * PAD],
                    in_=x8n_d[icc, :, r0 * PAD : r1 * PAD],
                )

            # Arrival-matched schedule. Per-queue transfers are serial
            # (~30 GB/s each) and the k conv consumes ~2 rows/us from
            # t~10.5us, so slices are ordered per queue by when their rows
            # are needed. SC clears its small weight transfers first, then
            # covers the rows-30..66 window where SY/GP fall behind;
            # occ0's v/q weights (9:27) land by ~63us (needed ~79/~146)
            # and occ1's (27:54) by ~92us (needed in phase B).
            up_xa(SY, 0, 0, 6)
            up_w(SY, 1, 3)
            up_xa(SY, 1, 6, 18)
            up_xa(SY, 0, 18, 30)
            up_xa(SY, 1, 42, 54)
            up_xa(SY, 0, 66, 78)
            up_xa(SY, 1, 78, 90)
            up_xa(SY, 0, 90, 102)
            up_xa(SY, 1, 102, 114)
            up_xa(SY, 0, 114, 126)
            up_xa(SY, 1, 126, 130)
            up_xa(GP, 1, 0, 6)
            up_w(GP, 3, 5)
            up_xa(GP, 0, 6, 18)
            up_xa(GP, 1, 18, 30)
            up_xa(GP, 0, 42, 54)
            up_xa(GP, 1, 66, 78)
            up_xa(GP, 0, 78, 90)
            up_xa(GP, 1, 90, 102)
            up_xa(GP, 0, 102, 114)
            up_xa(GP, 1, 114, 126)
            up_xa(GP, 0, 126, 130)
            up_w(SC, 0, 1)
            up_w(SC, 5, 7)
            up_w(SC, 7, 9)
            nc.scalar.dma_start(out=b_sb[:], in_=b_d)
            up_xa(SC, 0, 30, 42)
            up_xa(SC, 1, 30, 42)
            up_xa(SC, 0, 54, 66)
            up_xa(SC, 1, 54, 66)
            up_w(SC, 9, 27)
            up_w(SC, 27, 54)

            # xt borders zeroed once; interior filled by castT jobs
            for icc in range(2):
                vt = xt[:, icc, 0:PADSZ].rearrange("p (r c) -> p r c", c=PAD)
                nc.vector.memset(vt[:, 0, :], 0.0)
                nc.vector.memset(vt[:, PAD - 1, :], 0.0)
                nc.vector.memset(vt[:, :, 0:1], 0.0)
                nc.vector.memset(vt[:, :, PAD - 1 : PAD], 0.0)
            castT_jobs = []
            for s in range(16):
                for icc in range(2):

                    def castT(s=s, icc=icc):
                        va = xa[:, icc, 0:PADSZ].rearrange("p (r c) -> p r c", c=PAD)
                        vt = xt[:, icc, 0:PADSZ].rearrange("p (r c) -> p r c", c=PAD)
                        dst = vt[:, 1 : 1 + W, 1 + s * 8 : 9 + s * 8]
                        srcv = va[:, 1 + s * 8 : 9 + s * 8, 1 : 1 + W].rearrange(
                            "p h w -> p w h"
                        )
                        if (s * 2 + icc) % 2 == 0:
                            nc.scalar.activation(out=dst, in_=srcv, func=AF.Copy)
                        else:
                            nc.vector.tensor_copy(out=dst, in_=srcv)

                    castT_jobs.append(castT)

            # ---- HBM round-trip buffers: position-major [j, c, i] ----
            qt_dram = dram.tile([128, C, 128], dt.float8e4, tag="qt")
            k_dram = dram.tile([128, C, 128], dt.float8e4, tag="kd")
            v_dram = dram.tile([128, C, 128], dt.float8e4, tag="vd")
            cv_dram = [k_dram, v_dram, qt_dram]  # cvslot order: k, v, q

            evq = [SY, GP]

            def conv_group(occ, cvslot, gi, chunks):
                # one PSUM bank per chunk; shifts share each weight load.
                # Flat [128, 2, nr*PAD] rhs runs keep the PE at 1 col/cycle
                # (a 4-D valid-cols-only rhs measured ~30 cycles/row of
                # restart penalty); the seam junk is simply never evacuated.
                src = xt if cvslot == 2 else xa
                ps = [
                    psum_c.tile([128, 3, PAD], dt.float32, tag="psc", name=f"psc{ci}")
                    for ci in range(len(chunks))
                ]
                for kk in range(9):
                    dy, dx = kk // 3, kk % 3
                    w3 = w_sb[:, occ * 27 + cvslot * 9 + kk, :].rearrange(
                        "p (two o) -> p two o", two=2
                    )
                    for ci, (r0, nr) in enumerate(chunks):
                        s0 = (r0 + dy) * PAD + dx
                        nc.tensor.matmul(
                            ps[ci][:, 0:nr, :],
                            lhsT=w3,
                            rhs=src[:, :, s0 : s0 + nr * PAD],
                            start=(kk == 0),
                            stop=(kk == 8),
                            perf_mode=DR,
                        )
                # merged pair evacuation: one fp8 tile, one store DMA
                rows = sum(nr for _, nr in chunks)
                ev = evac.tile([128, 6, 128], dt.float8e4, tag="ev")
                ro = 0
                for ci, (r0, nr) in enumerate(chunks):
                    nc.scalar.activation(
                        out=ev[:, ro : ro + nr, :],
                        in_=ps[ci][:, 0:nr, 0:128],
                        func=AF.Identity if cvslot == 1 else AF.Relu,
                        bias=b_sb[:, occ * 3 + cvslot : occ * 3 + cvslot + 1],
                        scale=1.0 / WSCALE,
                    )
                    ro += nr
                r0a = chunks[0][0]
                evq[gi % 2].dma_start(
                    out=cv_dram[cvslot][
                        r0a : r0a + rows, occ * 128 : (occ + 1) * 128, :
                    ].rearrange("j c i -> c j i"),
                    in_=ev[:, 0:rows, :],
                )

            # ---- attention: 8-channel groups, front/back split ----
            # front = S matmuls + exp + row-sums + ones-matmul Z broadcast
            # (needs q, k); back = Y matmuls + fused (y/Z + x) + store
            # (needs v and the x residual slice). Fronts of block occ run
            # as soon as that block's q and k convs are done; backs only
            # need v, so the phase-C tail is just Y+scale+store.
            def att_load_qk(occ, g0):
                c0 = occ * 128 + g0
                qt8 = qload.tile([128, 8, 128], dt.float8e4, tag="qt8")
                nc.sync.dma_start(out=qt8[:], in_=qt_dram[:, c0 : c0 + 8, :])
                k8 = kload.tile([128, 8, 128], dt.float8e4, tag="k8")
                nc.gpsimd.dma_start(out=k8[:], in_=k_dram[:, c0 : c0 + 8, :])
                # the x residual slice has no conv dependency: load it here
                # (phase B) so the phase-C tail only moves v8 + y stores
                xr8 = xrload.tile([128, 8, 128], dt.bfloat16, tag="xr8")
                (nc.sync if (g0 // 8) % 2 else nc.gpsimd).dma_start(
                    out=xr8[:], in_=xbf_d[:, c0 : c0 + 8, :]
                )
                return qt8, k8, xr8

            vq = [nc.sync, nc.gpsimd, nc.scalar]

            def att_load_v(occ, g0, phc=False):
                c0 = occ * 128 + g0
                v8 = vload.tile([128, 8, 128], dt.float8e4, tag="v8")
                q = vq[(g0 // 8) % 3] if phc else nc.sync
                q.dma_start(out=v8[:], in_=v_dram[:, c0 : c0 + 8, :])
                return v8

            def att_front(occ, g0, qk, pool):
                qt8, k8 = qk
                ps_s = [
                    pool[0].tile([128, 4, 128], dt.float32, tag=pool[1], name=f"ps_s{i}")
                    for i in range(2)
                ]
                for j in range(8):
                    nc.tensor.matmul(
                        ps_s[j // 4][:, j % 4, :],
                        lhsT=k8[:, j, :],
                        rhs=qt8[:, j, :],
                        start=True,
                        stop=True,
                    )
                p8 = att.tile([128, 8, 128], dt.bfloat16, tag="p8")
                nc.scalar.activation(out=p8[:, 0:4, :], in_=ps_s[0][:], func=AF.Exp)
                nc.scalar.activation(out=p8[:, 4:8, :], in_=ps_s[1][:], func=AF.Exp)
                cs8 = stat.tile([128, 8], dt.bfloat16, tag="cs8")
                # bf16 partial sums feed a bf16 ones-matmul (halves its
                # LDWEIGHTS); ~0.4% on Z, well inside the error budget
                with nc.allow_low_precision(reason="bf16 softmax-sum broadcast"):
                    nc.vector.reduce_sum(cs8[:], p8[:], axis=mybir.AxisListType.X)
                ps_z = psum_z.tile([128, 8], dt.float32, tag="psz")
                nc.tensor.matmul(
                    ps_z[:], lhsT=ones_bf[:], rhs=cs8[:], start=True, stop=True
                )
                rec8 = stat.tile([128, 8], dt.float32, tag="rec8")
                nc.vector.reciprocal(rec8[:], ps_z[:])
                # pre-scale P by 1/Z now (hidden under conv matmuls) so the
                # back needs no per-channel scalar — its residual add can be
                # batched per PSUM bank
                p8s = att2.tile([128, 8, 128], dt.bfloat16, tag="p8s")
                for j in range(8):
                    nc.vector.tensor_scalar_mul(
                        p8s[:, j, :], p8[:, j, :], rec8[:, j : j + 1]
                    )
                return p8s

            def att_back(occ, g0, v8, xr8, fr, pool, split_store, phc=False):
                p8s = fr
                c0 = occ * 128 + g0
                ps_y = [
                    pool[0].tile([128, 4, 128], dt.float32, tag=pool[1], name=f"ps_y{i}")
                    for i in range(2)
                ]
                for j in range(8):
                    nc.tensor.matmul(
                        ps_y[j // 4][:, j % 4, :],
                        lhsT=p8s[:, j, :],
                        rhs=v8[:, j, :],
                        start=True,
                        stop=True,
                    )
                out8 = outp.tile([128, 8, 128], dt.bfloat16, tag="out8")
                # P was pre-scaled by 1/Z, so just psum_y + x, one batched
                # add per PSUM bank (GpSimd cannot read PSUM -> DVE)
                for b in range(2):
                    nc.vector.tensor_tensor(
                        out=out8[:, 4 * b : 4 * b + 4, :],
                        in0=ps_y[b][:],
                        in1=xr8[:, 4 * b : 4 * b + 4, :],
                        op=ALU.add,
                    )
                if split_store:
                    sq = [SY, GP, SC, SY]
                    for h in range(4):
                        sq[h].dma_start(
                            out=y_d[:, c0 + 2 * h : c0 + 2 * h + 2, :],
                            in_=out8[:, 2 * h : 2 * h + 2, :],
                        )
                elif phc:
                    # scalar queue is compute-idle in phase C; halve each
                    # store's wall time by splitting it across two rings
                    i0 = g0 // 8
                    for h in range(2):
                        vq[(i0 + 1 + h) % 3].dma_start(
                            out=y_d[:, c0 + 4 * h : c0 + 4 * h + 4, :],
                            in_=out8[:, 4 * h : 4 * h + 4, :],
                        )
                else:
                    evq[(g0 // 8) % 2].dma_start(
                        out=y_d[:, c0 : c0 + 8, :], in_=out8[:]
                    )

            # ---- per-group step factories ----
            def group_steps(occ, pool):
                """Return [loadqk, loadv, front, back] thunks for each group."""
                out = []
                for g0 in range(0, 128, 8):
                    st = {}

                    def loadqk(g0=g0, st=st):
                        st["qk"] = att_load_qk(occ, g0)

                    def loadv(phc=False, g0=g0, st=st):
                        st["v"] = att_load_v(occ, g0, phc)

                    def front(g0=g0, st=st, pool=pool):
                        st["fr"] = att_front(occ, g0, st["qk"][:2], pool)

                    def back(pool_=None, split=False, phc=False, g0=g0, st=st, pool=pool):
                        att_back(
                            occ, g0, st["v"], st["qk"][2], st["fr"],
                            pool_ or pool, split, phc,
                        )

                    out.append((loadqk, loadv, front, back))
                return out

            # Phase A: block-0 convs (k, v, q). castT jobs run 2-per-slot
            # in the first 16 slots so xt is complete well before the q conv.
            slot_jobs = iter(
                [[castT_jobs[2 * i], castT_jobs[2 * i + 1]] for i in range(16)]
            )
            gi = 0
            for cvslot in (0, 1, 2):
                for grp in GROUPS:
                    conv_group(0, cvslot, gi, grp)
                    gi += 1
                    for job in next(slot_jobs, []):
                        job()

            # Phase B: block-1 convs in q, k, v order. Block-0's full
            # attention (4 steps/group) spreads over all 66 slots; block-1
            # fronts (loadqk+front) run in the v-conv third once block-1's
            # q and k are done.
            b0 = group_steps(0, (psum_a, "psa"))
            b1 = group_steps(1, (psum_a, "psa"))
            b0_flat = [s for g in b0 for s in (g[0], g[1], g[2], g[3])]
            b1_fronts = [s for g in b1 for s in (g[0], g[2])]
            nslots = 3 * len(GROUPS)
            b0_done = b1_done = 0
            slot = 0
            for cvslot in (2, 0, 1):
                for grp in GROUPS:
                    conv_group(1, cvslot, gi, grp)
                    gi += 1
                    slot += 1
                    while b0_done < min((slot * 64) // nslots, len(b0_flat)):
                        b0_flat[b0_done]()
                        b0_done += 1
                    if slot > 2 * len(GROUPS):
                        # ~3 jobs per 2 slots: loads run a slot ahead of
                        # their fronts, and the last front issues ~6 conv
                        # groups before the v conv ends so its exp/sum/Z
                        # chain stays hidden under conv matmuls
                        vslot = slot - 2 * len(GROUPS)
                        while b1_done < min((vslot * 3) // 2 + 1, 32):
                            b1_fronts[b1_done]()
                            b1_done += 1
            for s in b0_flat[b0_done:]:
                s()
            for s in b1_fronts[b1_done:]:
                s()

            # Phase C: block-1 backs only (Y matmuls + scale/residual +
            # store), loads prefetched two groups ahead, PSUM pools
            # alternating, final stores split across queues.
            pools = [(psum_a, "psa"), (psum_c, "psc")]
            b1[0][1](phc=True)
            b1[1][1](phc=True)
            for i in range(16):
                b1[i][3](pool_=pools[i % 2], split=(i >= 14), phc=True)
                if i + 2 < 16:
                    b1[i + 2][1](phc=True)

    nc.compile()
    return nc


def _get_program():
    global _PROG
    if _PROG is None:
        _PROG = _build_program()
    return _PROG


def _pack_weights(Wq, Wk, Wv):
    # w_d[t, ic, icc*128 + oc], t = occ*27 + cvslot*9 + kk (cvslot: k,v,q).
    # The q conv runs on the TRANSPOSED image with the same (dy,dx) shift
    # arithmetic, so its taps must be packed transposed.
    out = np.zeros((54, 128, 256), np.float32)
    for cvslot, Wcv in ((0, Wk), (1, Wv), (2, np.asarray(Wq).transpose(0, 1, 3, 2))):
        a = np.asarray(Wcv, np.float32) * WSCALE  # [ocg, icg, dy, dx]
        a = a.transpose(2, 3, 1, 0).reshape(9, 2, 128, 2, 128)  # kk,icc,ic,occ,oc
        a = a.transpose(3, 0, 2, 1, 4).reshape(2, 9, 128, 256)  # occ,kk,ic,(icc oc)
        for occ in range(2):
            base = occ * 27 + cvslot * 9
            out[base : base + 9] = a[occ]
    return np.clip(out, -240, 240).astype(ml_dtypes.float8_e4m3)


def _pack_x8(xb):
    # xb [256, 128, 128] fp32 -> padded fp8 natural layout
    x8 = np.asarray(xb, np.float32).astype(ml_dtypes.float8_e4m3)
    x8 = x8.reshape(2, 128, H, W)
    nat = np.zeros((2, 128, PAD, PAD), ml_dtypes.float8_e4m3)
    nat[:, :, 1 : 1 + H, 1 : 1 + W] = x8
    full_n = np.zeros((2, 128, PADSTRIDE), ml_dtypes.float8_e4m3)
    full_n[:, :, :PADSZ] = nat.reshape(2, 128, PADSZ)
    return np.ascontiguousarray(full_n)


def _run(inputs, trace=False, trace_kwargs=None):
    from concourse.bass_utils import run_bass_kernel_spmd

    nc = _get_program()
    x = np.ascontiguousarray(np.asarray(inputs["x"], np.float32))
    wpack = _pack_weights(inputs["Wq"], inputs["Wk"], inputs["Wv"])
    bq = np.asarray(inputs["bq"], np.float32)
    bk = np.asarray(inputs["bk"], np.float32)
    bv = np.asarray(inputs["bv"], np.float32)
    # col = occ*3 + cvslot, cvslot order (k, v, q)
    bpack = np.stack(
        [bk[:128], bv[:128], bq[:128], bk[128:], bv[128:], bq[128:]], axis=1
    )
    bpack = np.ascontiguousarray(bpack, dtype=np.float32)  # [128, 6]

    in_maps = []
    for b in range(N_CORES):
        x8n = _pack_x8(x[b])
        xbf = np.ascontiguousarray(
            x[b].astype(ml_dtypes.bfloat16).transpose(1, 0, 2)
        )
        in_maps.append(
            {"xbf": xbf, "x8n": x8n, "wpack": wpack, "bpack": bpack}
        )
    last_err = None
    for attempt in range(3):
        try:
            res = run_bass_kernel_spmd(
                nc,
                in_maps,
                core_ids=list(range(N_CORES)),
                trace=trace,
                **(trace_kwargs or {}),
            )
            break
        except Exception as e:  # transient device/runtime hiccups
            last_err = e
            if attempt == 2:
                raise
            import time

            time.sleep(5.0)
    out = np.stack(
        [
            np.asarray(res.results[b]["y"], np.float32).transpose(1, 0, 2)
            for b in range(N_CORES)
        ],
        axis=0,
    )
    return out, res


def kernel(**inputs) -> np.ndarray:
    out, _ = _run(inputs, trace=False)
    return out


def kernel_traced(inputs):
    try:
        import axon_shim

        axon_shim.install()
    except Exception:
        pass
    out, res = _run(inputs, trace=True)
    return out, res
